# revision 26
# baseline (speedup 1.0000x reference)
"""ColorHistogramLoss Trainium2 kernel (8 NeuronCores, SPMD).

Math: soft histogram with Gaussian kernel, sigma = bin width (1/64).
    w(x, j) = exp(-(64*x - (j+0.5))^2 / 2)           [bin units]
            = (sqrt(pi)/2) * DErf((64*x - (j+0.5))/sqrt(2))
where DErf(t) = d/dt erf(t) = (2/sqrt(pi)) exp(-t^2) is a ScalarE LUT
function.  The global (2/sqrt(pi)) factor cancels in the per-channel
normalization (epsilon scaled to match).

Sharding: 24 channels total (pred 12 + target 12, channel = (tensor,b,c));
core i gets channels {3i, 3i+1, 3i+2}.  Each channel's 512*512 pixels are
laid out as [128 partitions, 2048].  One ScalarE activation instruction per
(channel, bin) computes the Gaussian AND its free-dim sum (accum_out);
TensorE reduces the 128 partitions with a ones-matmul; VectorE cumsums and
computes reciprocals of the totals; an AllGather shares the 24 per-channel
(cumsum, 1/total) rows; every core then computes the identical scalar loss.
"""

import math
from contextlib import ExitStack

import numpy as np

import concourse.bass as bass
import concourse.mybir as mybir
from concourse.bass_utils import run_bass_kernel_spmd

BINS = 64
N_CORES = 8
CH_PER_CORE = 3  # 24 channels / 8 cores
P = 128
G = 2048  # 512*512 / 128
PACK = BINS + 1  # per-channel payload: 64 cumsum values + 1 reciprocal
F32 = mybir.dt.float32

SCALE = 64.0 / math.sqrt(2.0)
INV_SQRT2 = 1.0 / math.sqrt(2.0)
# reference adds 1e-8 to the hist sum; our hist carries an extra 2/sqrt(pi)
EPS = (2.0 / math.sqrt(math.pi)) * 1e-8
AX = mybir.AxisListType
OP = mybir.AluOpType
AF = mybir.ActivationFunctionType


def build_nc():
    nc = bass.Bass(num_devices=N_CORES)

    x_ext = nc.declare_dram_parameter("x", [CH_PER_CORE, P, G], F32, isOutput=False)
    out_ext = nc.declare_dram_parameter("out", [1, 1], F32, isOutput=True)

    cdf_in = nc.dram_tensor("cdf_in", [1, CH_PER_CORE * PACK], F32)
    cdf_all = nc.dram_tensor(
        "cdf_all", [N_CORES, CH_PER_CORE * PACK], F32, addr_space="Shared"
    )

    bias_np = np.tile(
        (-(np.arange(BINS, dtype=np.float64) + 0.5) * INV_SQRT2).astype(np.float32),
        (P, 1),
    )
    bias_dram = nc.inline_tensor(bias_np, name="bias_const")

    with ExitStack() as stack:
        e = stack.enter_context
        xs = e(nc.sbuf_tensor("xs", [P, CH_PER_CORE * G], F32))
        scratch = e(nc.sbuf_tensor("scratch", [P, G], F32))
        acc = e(nc.sbuf_tensor("acc", [P, CH_PER_CORE * BINS], F32))
        ones_sb = e(nc.sbuf_tensor("ones", [P, 1], F32))
        biases_sb = e(nc.sbuf_tensor("biases", [P, BINS], F32))
        hrow = e(nc.sbuf_tensor("hrow", [1, CH_PER_CORE * BINS], F32))
        hsum = e(nc.sbuf_tensor("hsum", [1, CH_PER_CORE], F32))
        rinv = e(nc.sbuf_tensor("rinv", [1, CH_PER_CORE], F32))
        packed = e(nc.sbuf_tensor("packed", [1, CH_PER_CORE * PACK], F32))
        gp = e(nc.sbuf_tensor("gp", [12, PACK], F32))
        gt = e(nc.sbuf_tensor("gt", [12, PACK], F32))
        t1 = e(nc.sbuf_tensor("t1", [12, BINS], F32))
        t2 = e(nc.sbuf_tensor("t2", [12, BINS], F32))
        ra = e(nc.sbuf_tensor("ra", [12, 1], F32))
        loss_sb = e(nc.sbuf_tensor("loss", [1, 1], F32))
        ph = e(nc.psum_tensor("ph", [1, CH_PER_CORE * BINS], F32))
        pl = e(nc.psum_tensor("pl", [1, 1], F32))
        dma_sem = e(nc.semaphore("dma_sem"))
        act_sem = e(nc.semaphore("act_sem"))
        pe_sem = e(nc.semaphore("pe_sem"))
        dve_sem = e(nc.semaphore("dve_sem"))
        ones_sem = e(nc.semaphore("ones_sem"))
        cc_sem = e(nc.semaphore("cc_sem"))
        block = e(nc.Block())

        @block.sync
        def _(sync: bass.BassEngine):
            sync.dma_start(out=biases_sb[:, :], in_=bias_dram[:, :]).then_inc(
                dma_sem, 16
            )
            for c in range(CH_PER_CORE):
                sync.dma_start(
                    out=xs[:, c * G : (c + 1) * G], in_=x_ext[c, :, :]
                ).then_inc(dma_sem, 16)
            # packed (cumsums + reciprocals) -> collective input
            sync.wait_ge(dve_sem, 1)
            sync.dma_start(out=cdf_in[:, :], in_=packed[:, :]).then_inc(dma_sem, 16)
            # gathered payload -> SBUF; pred and target into separate tiles
            sync.wait_ge(cc_sem, 1)
            sync.dma_start(out=gp[:, :], in_=cdf_all[0:4, :]).then_inc(dma_sem, 16)
            sync.dma_start(out=gt[:, :], in_=cdf_all[4:8, :]).then_inc(dma_sem, 16)
            # final scalar -> output
            sync.wait_ge(act_sem, 2)
            sync.dma_start(out=out_ext[:, :], in_=loss_sb[:, :]).then_inc(dma_sem, 16)

        @block.scalar
        def _(scalar: bass.BassScalarEngine):
            scalar.wait_ge(dma_sem, 64)
            for c in range(CH_PER_CORE):
                xin = xs[:, c * G : (c + 1) * G]
                for j in range(BINS):
                    ins = scalar.activation(
                        scratch[:, :],
                        xin,
                        AF.Derivative_Erf,
                        bias=biases_sb[:, j : j + 1],
                        scale=SCALE,
                        accum_out=acc[:, c * BINS + j : c * BINS + j + 1],
                    )
            ins.then_inc(act_sem, 1)
            # final: loss = pl / 768
            scalar.wait_ge(pe_sem, 2)
            scalar.mul(loss_sb[:, :], pl[:, :], 1.0 / 768.0).then_inc(act_sem, 1)

        @block.vector
        def _(vector: bass.BassVectorEngine):
            vector.memset(ones_sb[:, :], 1.0).then_inc(ones_sem, 1)
            vector.wait_ge(pe_sem, 1)
            vector.tensor_copy(hrow[:, :], ph[:, :])
            # per-channel totals -> + eps -> reciprocal
            vector.tensor_reduce(
                hsum[:, :],
                hrow.ap().rearrange("p (c j) -> p c j", c=CH_PER_CORE),
                AX.X,
                OP.add,
            )
            vector.scalar_tensor_tensor(
                hsum[:, :], hsum[:, :], EPS, hsum[:, :], OP.add, OP.bypass
            )
            vector.reciprocal(rinv[:, :], hsum[:, :])
            # per-channel cumsum into the packed payload
            for c in range(CH_PER_CORE):
                vector.tensor_tensor_scan(
                    packed[:, c * PACK : c * PACK + BINS],
                    hrow[:, c * BINS : (c + 1) * BINS],
                    hrow[:, c * BINS : (c + 1) * BINS],
                    0.0,
                    OP.add,
                    OP.bypass,
                )
            # reciprocals into slot 64 of each channel payload
            vector.tensor_copy(
                packed[:, BINS :: PACK],
                rinv[:, :],
            ).then_inc(dve_sem, 1)
            # ----- loss stage (after the all-gather) -----
            vector.wait_ge(dma_sem, 112)
            vector.scalar_tensor_tensor(
                t1[:, :],
                gp[:, 0:BINS],
                gp[:, BINS : BINS + 1],
                gp[:, 0:BINS],
                OP.mult,
                OP.bypass,
            )
            vector.scalar_tensor_tensor(
                t2[:, :],
                gt[:, 0:BINS],
                gt[:, BINS : BINS + 1],
                t1[:, :],
                OP.mult,
                OP.subtract,
            )
            vector.tensor_reduce(
                ra[:, :], t2[:, :], AX.X, OP.add, apply_absolute_value=True
            ).then_inc(dve_sem, 1)

        @block.tensor
        def _(tensor: bass.BassTensorEngine):
            tensor.wait_ge(ones_sem, 1)
            tensor.wait_ge(act_sem, 1)
            tensor.matmul(
                ph[:, :], ones_sb[:, 0:1], acc[:, :], start=True, stop=True
            ).then_inc(pe_sem, 1)
            tensor.wait_ge(dve_sem, 2)
            tensor.matmul(
                pl[0:1, 0:1], ones_sb[0:12, 0:1], ra[0:12, 0:1], start=True, stop=True
            ).then_inc(pe_sem, 1)

        @block.gpsimd
        def _(gpsimd: bass.BassGpSimd):
            gpsimd.wait_ge(dma_sem, 80)
            gpsimd.collective_compute(
                "AllGather",
                OP.bypass,
                replica_groups=[list(range(N_CORES))],
                ins=[cdf_in.ap()],
                outs=[cdf_all.ap()],
            ).then_inc(cc_sem, 1)

    return nc


_NC_CACHE = None


def _get_nc():
    global _NC_CACHE
    if _NC_CACHE is None:
        _NC_CACHE = build_nc()
    return _NC_CACHE


def kernel(pred: np.ndarray, target: np.ndarray) -> np.ndarray:
    assert pred.shape == (4, 3, 512, 512) and target.shape == (4, 3, 512, 512)
    chans = np.concatenate(
        [
            np.ascontiguousarray(pred, dtype=np.float32).reshape(12, P, G),
            np.ascontiguousarray(target, dtype=np.float32).reshape(12, P, G),
        ],
        axis=0,
    )  # [24, 128, 2048]

    in_maps = [
        {"x": np.ascontiguousarray(chans[3 * i : 3 * i + 3])} for i in range(N_CORES)
    ]

    nc = _get_nc()
    res = run_bass_kernel_spmd(nc, in_maps, core_ids=list(range(N_CORES)))
    out = res.results[0]["out"]
    return np.asarray(out, dtype=np.float32).reshape(())


# revision 39
# speedup vs baseline: 1.1652x; 1.1652x over previous
"""ColorHistogramLoss Trainium2 kernel (8 NeuronCores, SPMD).

Math: soft histogram with Gaussian kernel, sigma = bin width (1/64).
    w(x, j) = exp(-(64*x - (j+0.5))^2 / 2)           [bin units]
            = (sqrt(pi)/2) * DErf((64*x - (j+0.5))/sqrt(2))
where DErf(t) = d/dt erf(t) = (2/sqrt(pi)) exp(-t^2) is a ScalarE LUT
function.  The global (2/sqrt(pi)) factor cancels in the per-channel
normalization (epsilon scaled to match).

Sharding: 24 channels total (pred 12 + target 12, channel = (tensor,b,c));
core i gets channels {3i, 3i+1, 3i+2}.  Each channel's 512*512 pixels are
laid out as [128 partitions, 2048].  One ScalarE activation instruction per
(channel, bin) computes the Gaussian AND its free-dim sum (accum_out);
TensorE reduces the 128 partitions with a ones-matmul; VectorE cumsums and
computes reciprocals of the totals; an AllGather shares the 24 per-channel
(cumsum, 1/total) rows; every core then computes the identical scalar loss.
"""

import math
from contextlib import ExitStack

import numpy as np

import concourse.bass as bass
import concourse.mybir as mybir
from concourse.bass_utils import run_bass_kernel_spmd

BINS = 64
N_CORES = 8
CH_PER_CORE = 3  # 24 channels / 8 cores
P = 128
G = 2048  # 512*512 / 128
PACK = BINS + 1  # per-channel payload: 64 cumsum values + 1 reciprocal
F32 = mybir.dt.float32

SCALE = 64.0 / math.sqrt(2.0)
INV_SQRT2 = 1.0 / math.sqrt(2.0)
# reference adds 1e-8 to the hist sum; our hist carries an extra 2/sqrt(pi)
EPS = (2.0 / math.sqrt(math.pi)) * 1e-8
AX = mybir.AxisListType
OP = mybir.AluOpType
AF = mybir.ActivationFunctionType


def build_nc():
    nc = bass.Bass(num_devices=N_CORES)

    x_ext = nc.declare_dram_parameter("x", [CH_PER_CORE, P, G], F32, isOutput=False)
    out_ext = nc.declare_dram_parameter("out", [1, 1], F32, isOutput=True)

    cdf_in = nc.dram_tensor("cdf_in", [1, CH_PER_CORE * PACK], F32)
    cdf_all = nc.dram_tensor(
        "cdf_all", [N_CORES, CH_PER_CORE * PACK], F32, addr_space="Shared"
    )

    bias_np = np.tile(
        (-(np.arange(BINS, dtype=np.float64) + 0.5) * INV_SQRT2).astype(np.float32),
        (P, 1),
    )
    bias_dram = nc.inline_tensor(bias_np, name="bias_const")

    with ExitStack() as stack:
        e = stack.enter_context
        xs = e(nc.sbuf_tensor("xs", [P, CH_PER_CORE * G], F32))
        scratch = e(nc.sbuf_tensor("scratch", [P, G], F32))
        acc = e(nc.sbuf_tensor("acc", [P, CH_PER_CORE * BINS], F32))
        ones_sb = e(nc.sbuf_tensor("ones", [P, 1], F32))
        biases_sb = e(nc.sbuf_tensor("biases", [P, BINS], F32))
        hrow = e(nc.sbuf_tensor("hrow", [1, CH_PER_CORE * BINS], F32))
        hsum = e(nc.sbuf_tensor("hsum", [1, CH_PER_CORE], F32))
        rinv = e(nc.sbuf_tensor("rinv", [1, CH_PER_CORE], F32))
        packed = e(nc.sbuf_tensor("packed", [1, CH_PER_CORE * PACK], F32))
        gp = e(nc.sbuf_tensor("gp", [12, PACK], F32))
        gt = e(nc.sbuf_tensor("gt", [12, PACK], F32))
        t1 = e(nc.sbuf_tensor("t1", [12, BINS], F32))
        t2 = e(nc.sbuf_tensor("t2", [12, BINS], F32))
        ra = e(nc.sbuf_tensor("ra", [12, 1], F32))
        loss_sb = e(nc.sbuf_tensor("loss", [1, 1], F32))
        ph = e(nc.psum_tensor("ph", [1, CH_PER_CORE * BINS], F32))
        pl = e(nc.psum_tensor("pl", [1, 1], F32))
        dma_sem = e(nc.semaphore("dma_sem"))
        act_sem = e(nc.semaphore("act_sem"))
        pe_sem = e(nc.semaphore("pe_sem"))
        dve_sem = e(nc.semaphore("dve_sem"))
        ones_sem = e(nc.semaphore("ones_sem"))
        cc_sem = e(nc.semaphore("cc_sem"))
        block = e(nc.Block())

        @block.sync
        def _(sync: bass.BassEngine):
            sync.dma_start(out=biases_sb[:, :], in_=bias_dram[:, :]).then_inc(
                dma_sem, 16
            )
            for c in range(CH_PER_CORE):
                sync.dma_start(
                    out=xs[:, c * G : (c + 1) * G], in_=x_ext[c, :, :]
                ).then_inc(dma_sem, 16)
            # packed (cumsums + reciprocals) -> collective input
            sync.wait_ge(dve_sem, 1)
            sync.dma_start(out=cdf_in[:, :], in_=packed[:, :]).then_inc(dma_sem, 16)
            # gathered payload -> SBUF; pred and target into separate tiles
            sync.wait_ge(cc_sem, 1)
            sync.dma_start(out=gp[:, :], in_=cdf_all[0:4, :]).then_inc(dma_sem, 16)
            sync.dma_start(out=gt[:, :], in_=cdf_all[4:8, :]).then_inc(dma_sem, 16)
            # final scalar -> output
            sync.wait_ge(act_sem, 2)
            sync.dma_start(out=out_ext[:, :], in_=loss_sb[:, :]).then_inc(dma_sem, 16)

        @block.scalar
        def _(scalar: bass.BassScalarEngine):
            scalar.wait_ge(dma_sem, 64)
            for c in range(CH_PER_CORE):
                xin = xs[:, c * G : (c + 1) * G]
                for j in range(BINS):
                    ins = scalar.activation(
                        scratch[:, :],
                        xin,
                        AF.Derivative_Erf,
                        bias=biases_sb[:, j : j + 1],
                        scale=SCALE,
                        accum_out=acc[:, c * BINS + j : c * BINS + j + 1],
                    )
            ins.then_inc(act_sem, 1)
            # final: loss = pl / 768
            scalar.wait_ge(pe_sem, 2)
            scalar.mul(loss_sb[:, :], pl[:, :], 1.0 / 768.0).then_inc(act_sem, 1)

        @block.vector
        def _(vector: bass.BassVectorEngine):
            vector.memset(ones_sb[:, :], 1.0).then_inc(ones_sem, 1)
            vector.wait_ge(pe_sem, 1)
            vector.tensor_copy(hrow[:, :], ph[:, :])
            # per-channel totals -> + eps -> reciprocal
            vector.tensor_reduce(
                hsum[:, :],
                hrow.ap().rearrange("p (c j) -> p c j", c=CH_PER_CORE),
                AX.X,
                OP.add,
            )
            vector.scalar_tensor_tensor(
                hsum[:, :], hsum[:, :], EPS, hsum[:, :], OP.add, OP.bypass
            )
            vector.reciprocal(rinv[:, :], hsum[:, :])
            # per-channel cumsum into the packed payload
            for c in range(CH_PER_CORE):
                vector.tensor_tensor_scan(
                    packed[:, c * PACK : c * PACK + BINS],
                    hrow[:, c * BINS : (c + 1) * BINS],
                    hrow[:, c * BINS : (c + 1) * BINS],
                    0.0,
                    OP.add,
                    OP.bypass,
                )
            # reciprocals into slot 64 of each channel payload
            vector.tensor_copy(
                packed[:, BINS :: PACK],
                rinv[:, :],
            ).then_inc(dve_sem, 1)
            # ----- loss stage (after the all-gather) -----
            vector.wait_ge(dma_sem, 112)
            vector.scalar_tensor_tensor(
                t1[:, :],
                gp[:, 0:BINS],
                gp[:, BINS : BINS + 1],
                gp[:, 0:BINS],
                OP.mult,
                OP.bypass,
            )
            vector.scalar_tensor_tensor(
                t2[:, :],
                gt[:, 0:BINS],
                gt[:, BINS : BINS + 1],
                t1[:, :],
                OP.mult,
                OP.subtract,
            )
            vector.tensor_reduce(
                ra[:, :], t2[:, :], AX.X, OP.add, apply_absolute_value=True
            ).then_inc(dve_sem, 1)

        @block.tensor
        def _(tensor: bass.BassTensorEngine):
            tensor.wait_ge(ones_sem, 1)
            tensor.wait_ge(act_sem, 1)
            tensor.matmul(
                ph[:, :], ones_sb[:, 0:1], acc[:, :], start=True, stop=True
            ).then_inc(pe_sem, 1)
            tensor.wait_ge(dve_sem, 2)
            tensor.matmul(
                pl[0:1, 0:1], ones_sb[0:12, 0:1], ra[0:12, 0:1], start=True, stop=True
            ).then_inc(pe_sem, 1)

        @block.gpsimd
        def _(gpsimd: bass.BassGpSimd):
            gpsimd.wait_ge(dma_sem, 80)
            gpsimd.collective_compute(
                "AllGather",
                OP.bypass,
                replica_groups=[list(range(N_CORES))],
                ins=[cdf_in.ap()],
                outs=[cdf_all.ap()],
            ).then_inc(cc_sem, 1)

    return nc


def _host_k_matrix():
    """Synthesis matrix Ksb [128, 129]: col 0 = -pi bias, cols 1+h*64+j = K.

    hist_j = sum_t Re[(2-d_t0) c_t e^{-i pi t/80} e^{-i 2 pi t j/80} S_t],
    t = 8*t1 + t2;  S from the raw 16x16 trig-product block.
    """
    T1 = T2 = 8
    Pb = 64 * 1.25
    K = np.zeros((2 * T1, 2 * T2, BINS), np.float64)
    j = np.arange(BINS)
    for t1 in range(T1):
        for t2 in range(T2):
            t = t1 * T2 + t2
            ct = (math.sqrt(2 * math.pi) / Pb) * math.exp(
                -0.5 * (2 * math.pi * t / Pb) ** 2
            )
            mult = 1.0 if t == 0 else 2.0
            w = mult * ct * np.exp(-1j * 2 * np.pi * t * (0.5 + j) / Pb)
            K[2 * t1, 2 * t2] = w.real
            K[2 * t1 + 1, 2 * t2 + 1] = -w.real
            K[2 * t1, 2 * t2 + 1] = -w.imag
            K[2 * t1 + 1, 2 * t2, :] = -w.imag
    consts = np.zeros((128, 129), np.float32)
    consts[:, 0] = 0.0  # Sin bias (frac args are already in [-0.5, 0.5])
    for p in range(128):
        for h in range(2):
            a = h * 8 + p // 16
            b = p % 16
            consts[p, 1 + h * 64 : 1 + (h + 1) * 64] = K[a, b]
    return consts


GS = 1024  # groups (of 128 pixels) per strip
NSTRIP_PER_CH = G // GS
NSTRIPS = CH_PER_CORE * NSTRIP_PER_CH
NPAIR = 14  # harmonics per strip needing trig: t2=1..7 (freq t) and t1=1..7 (freq 8t)
F16 = mybir.dt.float16


def build_nc_fourier():
    nc = bass.Bass(num_devices=N_CORES)

    x_ext = nc.declare_dram_parameter("x", [CH_PER_CORE, P, G], F32, isOutput=False)
    out_ext = nc.declare_dram_parameter("out", [1, 1], F32, isOutput=True)

    cdf_in = nc.dram_tensor("cdf_in", [1, CH_PER_CORE * PACK], F32)
    cdf_all = nc.dram_tensor(
        "cdf_all", [N_CORES, CH_PER_CORE * PACK], F32, addr_space="Shared"
    )
    sdram = nc.dram_tensor("sdram", [16, CH_PER_CORE, 16], F32)
    consts_dram = nc.inline_tensor(_host_k_matrix(), name="consts_k")

    # pair schedule: (t, A-or-B, plane-base)
    pairs = [(t2, "A", 2 * t2) for t2 in range(1, 8)] + [
        (8 * t1, "B", 2 * t1) for t1 in range(1, 8)
    ]

    with ExitStack() as stack:
        e = stack.enter_context
        xs = e(nc.sbuf_tensor("xs", [P, CH_PER_CORE * G], F32))
        consts = e(nc.sbuf_tensor("consts", [P, 129], F32))
        # trig planes, double buffered: [A(16 planes) | B(16 planes)] * GS
        plA = [e(nc.sbuf_tensor(f"plA{b}", [P, 16 * GS], F16)) for b in range(2)]
        plB = [e(nc.sbuf_tensor(f"plB{b}", [P, 16 * GS], F16)) for b in range(2)]
        args = e(nc.sbuf_tensor("args", [P, 4 * GS], F32))  # fr/frc ring, 2 pairs
        irnd = e(nc.sbuf_tensor("irnd", [P, 2 * GS], mybir.dt.int32))
        ones_sb = e(nc.sbuf_tensor("ones", [P, 1], F32))
        S_sb = e(nc.sbuf_tensor("S_sb", [16, CH_PER_CORE * 16], F32))
        svec = e(nc.sbuf_tensor("svec", [P, 2 * CH_PER_CORE], F32))
        hsum = e(nc.sbuf_tensor("hsum", [CH_PER_CORE, 1], F32))
        rinv = e(nc.sbuf_tensor("rinv", [CH_PER_CORE, 1], F32))
        packed = e(nc.sbuf_tensor("packed", [CH_PER_CORE, PACK], F32))
        gp = e(nc.sbuf_tensor("gp", [12, PACK], F32))
        gt = e(nc.sbuf_tensor("gt", [12, PACK], F32))
        t1s = e(nc.sbuf_tensor("t1s", [12, BINS], F32))
        t2s = e(nc.sbuf_tensor("t2s", [12, BINS], F32))
        ra = e(nc.sbuf_tensor("ra", [12, 1], F32))
        loss_sb = e(nc.sbuf_tensor("loss", [1, 1], F32))
        ps = e(nc.psum_tensor("ps", [16, CH_PER_CORE * 16], F32))
        ph2 = e(nc.psum_tensor("ph2", [CH_PER_CORE, BINS], F32))
        pl = e(nc.psum_tensor("pl", [1, 1], F32))
        dma_sem = e(nc.semaphore("dma_sem"))
        dve_pair = e(nc.semaphore("dve_pair"))
        act_pair = e(nc.semaphore("act_pair"))
        pe_strip = e(nc.semaphore("pe_strip"))
        pe_sem = e(nc.semaphore("pe_sem"))
        dve_sem = e(nc.semaphore("dve_sem"))
        act_sem = e(nc.semaphore("act_sem"))
        cc_sem = e(nc.semaphore("cc_sem"))
        block = e(nc.Block())

        def strip_x(s):
            c, si = divmod(s, NSTRIP_PER_CH)
            return xs[:, c * G + si * GS : c * G + (si + 1) * GS]

        @block.sync
        def _(sync: bass.BassEngine):
            sync.dma_start(out=consts[:, :], in_=consts_dram[:, :]).then_inc(
                dma_sem, 16
            )
            for c in range(CH_PER_CORE):
                sync.dma_start(
                    out=xs[:, c * G : (c + 1) * G], in_=x_ext[c, :, :]
                ).then_inc(dma_sem, 16)
            # S block -> DRAM -> svec (partition reshape)
            sync.wait_ge(dve_sem, 1)
            sync.dma_start(out=sdram[:, :, :], in_=S_sb[:, :]).then_inc(dma_sem, 16)
            for c in range(CH_PER_CORE):
                for h in range(2):
                    sync.dma_start(
                        out=svec[:, 2 * c + h : 2 * c + h + 1],
                        in_=sdram[h * 8 : (h + 1) * 8, c, :],
                    ).then_inc(dma_sem, 16)
            # packed cdfs -> collective input
            sync.wait_ge(dve_sem, 2)
            sync.dma_start(out=cdf_in[:, :], in_=packed[:, :]).then_inc(dma_sem, 16)
            # gathered payload
            sync.wait_ge(cc_sem, 1)
            sync.dma_start(out=gp[:, :], in_=cdf_all[0:4, :]).then_inc(dma_sem, 16)
            sync.dma_start(out=gt[:, :], in_=cdf_all[4:8, :]).then_inc(dma_sem, 16)
            # final scalar
            sync.wait_ge(act_sem, 1)
            sync.dma_start(out=out_ext[:, :], in_=loss_sb[:, :]).then_inc(dma_sem, 16)

        @block.vector
        def _(vector: bass.BassVectorEngine):
            vector.memset(ones_sb[:, :], 1.0)
            # constant t=0 planes: cos=1, sin=0
            for b in range(2):
                vector.memset(plA[b][:, 0:GS], 1.0)
                vector.memset(plA[b][:, GS : 2 * GS], 0.0)
                vector.memset(plB[b][:, 0:GS], 1.0)
                vector.memset(plB[b][:, GS : 2 * GS], 0.0)
            vector.wait_ge(dma_sem, 64)
            for s in range(NSTRIPS):
                xin = strip_x(s)
                for k, (t, _, _) in enumerate(pairs):
                    kg = s * NPAIR + k
                    if kg >= 2:
                        vector.wait_ge(act_pair, kg - 1)
                    slot = (kg % 2) * 2
                    fr = args[:, slot * GS : (slot + 1) * GS]
                    frc = args[:, (slot + 1) * GS : (slot + 2) * GS]
                    ir = irnd[:, (kg % 2) * GS : (kg % 2 + 1) * GS]
                    # fr = t*0.8*x - round(t*0.8*x)  in [-0.5, 0.5]
                    vector.tensor_scalar(ir, xin, t * 0.8, None, OP.mult)
                    vector.scalar_tensor_tensor(
                        fr, xin, t * 0.8, ir, OP.mult, OP.subtract
                    )
                    # frc = (fr+0.25) - round(fr+0.25): phase shifted by +pi/2
                    vector.tensor_scalar(ir, fr, 0.25, None, OP.add)
                    vector.scalar_tensor_tensor(
                        frc, fr, 0.25, ir, OP.add, OP.subtract
                    ).then_inc(dve_pair, 1)
            # ---- after PE S-accumulation: normalize + cumsum + pack ----
            vector.wait_ge(pe_strip, NSTRIPS)
            vector.tensor_copy(S_sb[:, :], ps[:, :]).then_inc(dve_sem, 1)
            vector.wait_ge(pe_sem, 1)
            vector.tensor_reduce(hsum[:, :], ph2[:, :], AX.X, OP.add)
            vector.scalar_tensor_tensor(
                hsum[:, :], hsum[:, :], 1e-8, hsum[:, :], OP.add, OP.bypass
            )
            vector.reciprocal(rinv[:, :], hsum[:, :])
            vector.tensor_tensor_scan(
                packed[:, 0:BINS],
                ph2[:, :],
                t1s[0:CH_PER_CORE, :],
                0.0,
                OP.add,
                OP.bypass,
            )
            vector.tensor_copy(packed[:, BINS : BINS + 1], rinv[:, :]).then_inc(
                dve_sem, 1
            )
            # ---- loss stage ----
            vector.wait_ge(dma_sem, 224)
            vector.scalar_tensor_tensor(
                t1s[:, :],
                gp[:, 0:BINS],
                gp[:, BINS : BINS + 1],
                gp[:, 0:BINS],
                OP.mult,
                OP.bypass,
            )
            vector.scalar_tensor_tensor(
                t2s[:, :],
                gt[:, 0:BINS],
                gt[:, BINS : BINS + 1],
                t1s[:, :],
                OP.mult,
                OP.subtract,
            )
            vector.tensor_reduce(
                ra[:, :], t2s[:, :], AX.X, OP.add, apply_absolute_value=True
            ).then_inc(dve_sem, 1)

        @block.scalar
        def _(scalar: bass.BassScalarEngine):
            scalar.wait_ge(dma_sem, 16)
            TWO_PI = 2.0 * math.pi
            for s in range(NSTRIPS):
                b = s % 2
                if s >= 2:
                    scalar.wait_ge(pe_strip, s - 1)
                for k, (t, which, q) in enumerate(pairs):
                    kg = s * NPAIR + k
                    scalar.wait_ge(dve_pair, kg + 1)
                    slot = (kg % 2) * 2
                    fr = args[:, slot * GS : (slot + 1) * GS]
                    frc = args[:, (slot + 1) * GS : (slot + 2) * GS]
                    pl_ = plA[b] if which == "A" else plB[b]
                    # -cos into plane q, -sin into plane q+1
                    scalar.activation(
                        pl_[:, q * GS : (q + 1) * GS],
                        frc,
                        AF.Sin,
                        bias=consts[:, 0:1],
                        scale=TWO_PI,
                    )
                    scalar.activation(
                        pl_[:, (q + 1) * GS : (q + 2) * GS],
                        fr,
                        AF.Sin,
                        bias=consts[:, 0:1],
                        scale=TWO_PI,
                    ).then_inc(act_pair, 1)
            # final: loss = pl / 768
            scalar.wait_ge(pe_sem, 2)
            scalar.mul(loss_sb[:, :], pl[:, :], 1.0 / 768.0).then_inc(act_sem, 1)

        @block.tensor
        def _(tensor: bass.BassTensorEngine):
            for s in range(NSTRIPS):
                b = s % 2
                c, si = divmod(s, NSTRIP_PER_CH)
                tensor.wait_ge(act_pair, NPAIR * (s + 1))
                for g in range(GS):
                    ins = tensor.matmul(
                        ps[:, 16 * c : 16 * (c + 1)],
                        plB[b][:, g :: GS],
                        plA[b][:, g :: GS],
                        start=(si == 0 and g == 0),
                        stop=(si == NSTRIP_PER_CH - 1 and g == GS - 1),
                    )
                ins.then_inc(pe_strip, 1)
            # synthesis: ph2[c, j] = sum_h sum_p svec[p, 2c+h] * K[p, h*64+j]
            tensor.wait_ge(dma_sem, 176)
            for h in range(2):
                ins = tensor.matmul(
                    ph2[:, :],
                    svec[:, h :: 2],
                    consts[:, 1 + h * BINS : 1 + (h + 1) * BINS],
                    start=(h == 0),
                    stop=(h == 1),
                )
            ins.then_inc(pe_sem, 1)
            tensor.wait_ge(dve_sem, 3)
            tensor.matmul(
                pl[0:1, 0:1], ones_sb[0:12, 0:1], ra[0:12, 0:1], start=True, stop=True
            ).then_inc(pe_sem, 1)  # pe_sem reaches 2

        @block.gpsimd
        def _(gpsimd: bass.BassGpSimd):
            gpsimd.wait_ge(dma_sem, 192)
            gpsimd.collective_compute(
                "AllGather",
                OP.bypass,
                replica_groups=[list(range(N_CORES))],
                ins=[cdf_in.ap()],
                outs=[cdf_all.ap()],
            ).then_inc(cc_sem, 1)

    return nc


import os

_VERSION = int(os.environ.get("BASS_HIST_V", "2"))
_NC_CACHE = None


def _get_nc():
    global _NC_CACHE
    if _NC_CACHE is None:
        _NC_CACHE = build_nc_fourier() if _VERSION == 2 else build_nc()
    return _NC_CACHE


def kernel(pred: np.ndarray, target: np.ndarray) -> np.ndarray:
    assert pred.shape == (4, 3, 512, 512) and target.shape == (4, 3, 512, 512)
    chans = np.concatenate(
        [
            np.ascontiguousarray(pred, dtype=np.float32).reshape(12, P, G),
            np.ascontiguousarray(target, dtype=np.float32).reshape(12, P, G),
        ],
        axis=0,
    )  # [24, 128, 2048]

    in_maps = [
        {"x": np.ascontiguousarray(chans[3 * i : 3 * i + 3])} for i in range(N_CORES)
    ]

    nc = _get_nc()
    res = run_bass_kernel_spmd(nc, in_maps, core_ids=list(range(N_CORES)))
    out = res.results[0]["out"]
    return np.asarray(out, dtype=np.float32).reshape(())


# revision 48
# speedup vs baseline: 1.4220x; 1.2204x over previous
"""ColorHistogramLoss Trainium2 kernel (8 NeuronCores, SPMD).

Math: soft histogram with Gaussian kernel, sigma = bin width (1/64).
    w(x, j) = exp(-(64*x - (j+0.5))^2 / 2)           [bin units]
            = (sqrt(pi)/2) * DErf((64*x - (j+0.5))/sqrt(2))
where DErf(t) = d/dt erf(t) = (2/sqrt(pi)) exp(-t^2) is a ScalarE LUT
function.  The global (2/sqrt(pi)) factor cancels in the per-channel
normalization (epsilon scaled to match).

Sharding: 24 channels total (pred 12 + target 12, channel = (tensor,b,c));
core i gets channels {3i, 3i+1, 3i+2}.  Each channel's 512*512 pixels are
laid out as [128 partitions, 2048].  One ScalarE activation instruction per
(channel, bin) computes the Gaussian AND its free-dim sum (accum_out);
TensorE reduces the 128 partitions with a ones-matmul; VectorE cumsums and
computes reciprocals of the totals; an AllGather shares the 24 per-channel
(cumsum, 1/total) rows; every core then computes the identical scalar loss.
"""

import math
from contextlib import ExitStack

import numpy as np

import concourse.bass as bass
import concourse.mybir as mybir
from concourse.bass_utils import run_bass_kernel_spmd

BINS = 64
N_CORES = 8
CH_PER_CORE = 3  # 24 channels / 8 cores
P = 128
G = 2048  # 512*512 / 128
PACK = BINS + 1  # per-channel payload: 64 cumsum values + 1 reciprocal
F32 = mybir.dt.float32

SCALE = 64.0 / math.sqrt(2.0)
INV_SQRT2 = 1.0 / math.sqrt(2.0)
# reference adds 1e-8 to the hist sum; our hist carries an extra 2/sqrt(pi)
EPS = (2.0 / math.sqrt(math.pi)) * 1e-8
AX = mybir.AxisListType
OP = mybir.AluOpType
AF = mybir.ActivationFunctionType


def build_nc():
    nc = bass.Bass(num_devices=N_CORES)

    x_ext = nc.declare_dram_parameter("x", [CH_PER_CORE, P, G], F32, isOutput=False)
    out_ext = nc.declare_dram_parameter("out", [1, 1], F32, isOutput=True)

    cdf_in = nc.dram_tensor("cdf_in", [1, CH_PER_CORE * PACK], F32)
    cdf_all = nc.dram_tensor(
        "cdf_all", [N_CORES, CH_PER_CORE * PACK], F32, addr_space="Shared"
    )

    bias_np = np.tile(
        (-(np.arange(BINS, dtype=np.float64) + 0.5) * INV_SQRT2).astype(np.float32),
        (P, 1),
    )
    bias_dram = nc.inline_tensor(bias_np, name="bias_const")

    with ExitStack() as stack:
        e = stack.enter_context
        xs = e(nc.sbuf_tensor("xs", [P, CH_PER_CORE * G], F32))
        scratch = e(nc.sbuf_tensor("scratch", [P, G], F32))
        acc = e(nc.sbuf_tensor("acc", [P, CH_PER_CORE * BINS], F32))
        ones_sb = e(nc.sbuf_tensor("ones", [P, 1], F32))
        biases_sb = e(nc.sbuf_tensor("biases", [P, BINS], F32))
        hrow = e(nc.sbuf_tensor("hrow", [1, CH_PER_CORE * BINS], F32))
        hsum = e(nc.sbuf_tensor("hsum", [1, CH_PER_CORE], F32))
        rinv = e(nc.sbuf_tensor("rinv", [1, CH_PER_CORE], F32))
        packed = e(nc.sbuf_tensor("packed", [1, CH_PER_CORE * PACK], F32))
        gp = e(nc.sbuf_tensor("gp", [12, PACK], F32))
        gt = e(nc.sbuf_tensor("gt", [12, PACK], F32))
        t1 = e(nc.sbuf_tensor("t1", [12, BINS], F32))
        t2 = e(nc.sbuf_tensor("t2", [12, BINS], F32))
        ra = e(nc.sbuf_tensor("ra", [12, 1], F32))
        loss_sb = e(nc.sbuf_tensor("loss", [1, 1], F32))
        ph = e(nc.psum_tensor("ph", [1, CH_PER_CORE * BINS], F32))
        pl = e(nc.psum_tensor("pl", [1, 1], F32))
        dma_sem = e(nc.semaphore("dma_sem"))
        act_sem = e(nc.semaphore("act_sem"))
        pe_sem = e(nc.semaphore("pe_sem"))
        dve_sem = e(nc.semaphore("dve_sem"))
        ones_sem = e(nc.semaphore("ones_sem"))
        cc_sem = e(nc.semaphore("cc_sem"))
        block = e(nc.Block())

        @block.sync
        def _(sync: bass.BassEngine):
            sync.dma_start(out=biases_sb[:, :], in_=bias_dram[:, :]).then_inc(
                dma_sem, 16
            )
            for c in range(CH_PER_CORE):
                sync.dma_start(
                    out=xs[:, c * G : (c + 1) * G], in_=x_ext[c, :, :]
                ).then_inc(dma_sem, 16)
            # packed (cumsums + reciprocals) -> collective input
            sync.wait_ge(dve_sem, 1)
            sync.dma_start(out=cdf_in[:, :], in_=packed[:, :]).then_inc(dma_sem, 16)
            # gathered payload -> SBUF; pred and target into separate tiles
            sync.wait_ge(cc_sem, 1)
            sync.dma_start(out=gp[:, :], in_=cdf_all[0:4, :]).then_inc(dma_sem, 16)
            sync.dma_start(out=gt[:, :], in_=cdf_all[4:8, :]).then_inc(dma_sem, 16)
            # final scalar -> output
            sync.wait_ge(act_sem, 2)
            sync.dma_start(out=out_ext[:, :], in_=loss_sb[:, :]).then_inc(dma_sem, 16)

        @block.scalar
        def _(scalar: bass.BassScalarEngine):
            scalar.wait_ge(dma_sem, 64)
            for c in range(CH_PER_CORE):
                xin = xs[:, c * G : (c + 1) * G]
                for j in range(BINS):
                    ins = scalar.activation(
                        scratch[:, :],
                        xin,
                        AF.Derivative_Erf,
                        bias=biases_sb[:, j : j + 1],
                        scale=SCALE,
                        accum_out=acc[:, c * BINS + j : c * BINS + j + 1],
                    )
            ins.then_inc(act_sem, 1)
            # final: loss = pl / 768
            scalar.wait_ge(pe_sem, 2)
            scalar.mul(loss_sb[:, :], pl[:, :], 1.0 / 768.0).then_inc(act_sem, 1)

        @block.vector
        def _(vector: bass.BassVectorEngine):
            vector.memset(ones_sb[:, :], 1.0).then_inc(ones_sem, 1)
            vector.wait_ge(pe_sem, 1)
            vector.tensor_copy(hrow[:, :], ph[:, :])
            # per-channel totals -> + eps -> reciprocal
            vector.tensor_reduce(
                hsum[:, :],
                hrow.ap().rearrange("p (c j) -> p c j", c=CH_PER_CORE),
                AX.X,
                OP.add,
            )
            vector.scalar_tensor_tensor(
                hsum[:, :], hsum[:, :], EPS, hsum[:, :], OP.add, OP.bypass
            )
            vector.reciprocal(rinv[:, :], hsum[:, :])
            # per-channel cumsum into the packed payload
            for c in range(CH_PER_CORE):
                vector.tensor_tensor_scan(
                    packed[:, c * PACK : c * PACK + BINS],
                    hrow[:, c * BINS : (c + 1) * BINS],
                    hrow[:, c * BINS : (c + 1) * BINS],
                    0.0,
                    OP.add,
                    OP.bypass,
                )
            # reciprocals into slot 64 of each channel payload
            vector.tensor_copy(
                packed[:, BINS :: PACK],
                rinv[:, :],
            ).then_inc(dve_sem, 1)
            # ----- loss stage (after the all-gather) -----
            vector.wait_ge(dma_sem, 112)
            vector.scalar_tensor_tensor(
                t1[:, :],
                gp[:, 0:BINS],
                gp[:, BINS : BINS + 1],
                gp[:, 0:BINS],
                OP.mult,
                OP.bypass,
            )
            vector.scalar_tensor_tensor(
                t2[:, :],
                gt[:, 0:BINS],
                gt[:, BINS : BINS + 1],
                t1[:, :],
                OP.mult,
                OP.subtract,
            )
            vector.tensor_reduce(
                ra[:, :], t2[:, :], AX.X, OP.add, apply_absolute_value=True
            ).then_inc(dve_sem, 1)

        @block.tensor
        def _(tensor: bass.BassTensorEngine):
            tensor.wait_ge(ones_sem, 1)
            tensor.wait_ge(act_sem, 1)
            tensor.matmul(
                ph[:, :], ones_sb[:, 0:1], acc[:, :], start=True, stop=True
            ).then_inc(pe_sem, 1)
            tensor.wait_ge(dve_sem, 2)
            tensor.matmul(
                pl[0:1, 0:1], ones_sb[0:12, 0:1], ra[0:12, 0:1], start=True, stop=True
            ).then_inc(pe_sem, 1)

        @block.gpsimd
        def _(gpsimd: bass.BassGpSimd):
            gpsimd.wait_ge(dma_sem, 80)
            gpsimd.collective_compute(
                "AllGather",
                OP.bypass,
                replica_groups=[list(range(N_CORES))],
                ins=[cdf_in.ap()],
                outs=[cdf_all.ap()],
            ).then_inc(cc_sem, 1)

    return nc


T1G = 6  # t1 grid (B side), harmonics at t = T2G*t1
T2G = 7  # t2 grid (A side)
NA = 2 * T2G  # A planes
NB = 2 * T1G  # B planes
BPAD = 16  # padded b-stride for the S-vector partition mapping


def _host_k_matrix():
    """Synthesis matrix Ksb [128, 129]: col 0 = Sin bias (0), cols 1+h*64+j = K.

    hist_j = sum_t Re[(2-d_t0) c_t e^{-i pi t/Pb} e^{-i 2 pi t j/Pb} S_t],
    t = T2G*t1 + t2;  S from the raw [2*T1G, 2*T2G] trig-product block,
    vectorized as v = a*BPAD + b (a = S row, b = S col), halves h of 128.
    """
    Pb = 64 * 1.25
    K = np.zeros((2 * T1G, BPAD, BINS), np.float64)
    j = np.arange(BINS)
    for t1 in range(T1G):
        for t2 in range(T2G):
            t = t1 * T2G + t2
            ct = (math.sqrt(2 * math.pi) / Pb) * math.exp(
                -0.5 * (2 * math.pi * t / Pb) ** 2
            )
            mult = 1.0 if t == 0 else 2.0
            w = mult * ct * np.exp(-1j * 2 * np.pi * t * (0.5 + j) / Pb)
            K[2 * t1, 2 * t2] = w.real
            K[2 * t1 + 1, 2 * t2 + 1] = -w.real
            K[2 * t1, 2 * t2 + 1] = -w.imag
            K[2 * t1 + 1, 2 * t2, :] = -w.imag
    consts = np.zeros((128, 129), np.float32)
    for p in range(128):
        for h in range(2):
            a = h * 8 + p // BPAD
            b = p % BPAD
            if a < 2 * T1G:
                consts[p, 1 + h * 64 : 1 + (h + 1) * 64] = K[a, b]
    return consts


GS = 1024  # groups (of 128 pixels) per strip
NSTRIP_PER_CH = G // GS
NSTRIPS = CH_PER_CORE * NSTRIP_PER_CH
NPAIR = (T1G - 1) + (T2G - 1)  # harmonic pairs needing trig per strip
F16 = mybir.dt.float16


def build_nc_fourier():
    nc = bass.Bass(num_devices=N_CORES)

    x_ext = nc.declare_dram_parameter("x", [CH_PER_CORE, P, G], F32, isOutput=False)
    out_ext = nc.declare_dram_parameter("out", [1, 1], F32, isOutput=True)

    cdf_in = nc.dram_tensor("cdf_in", [1, CH_PER_CORE * PACK], F32)
    cdf_all = nc.dram_tensor(
        "cdf_all", [N_CORES, CH_PER_CORE * PACK], F32, addr_space="Shared"
    )
    sdram = nc.dram_tensor("sdram", [2 * T1G, CH_PER_CORE, BPAD], F32)
    consts_dram = nc.inline_tensor(_host_k_matrix(), name="consts_k")

    # pair schedule: (t, A-or-B, plane-base)
    pairs = [(t2, "A", 2 * t2) for t2 in range(1, T2G)] + [
        (T2G * t1, "B", 2 * t1) for t1 in range(1, T1G)
    ]

    with ExitStack() as stack:
        e = stack.enter_context
        xs = e(nc.sbuf_tensor("xs", [P, CH_PER_CORE * G], F32))
        consts = e(nc.sbuf_tensor("consts", [P, 129], F32))
        # trig planes, double buffered: [A(16 planes) | B(16 planes)] * GS
        plA = [e(nc.sbuf_tensor(f"plA{b}", [P, NA * GS], F16)) for b in range(2)]
        plB = [e(nc.sbuf_tensor(f"plB{b}", [P, NB * GS], F16)) for b in range(2)]
        args = e(nc.sbuf_tensor("args", [P, 4 * GS], F32))  # fr/frc ring, 2 pairs
        irnd = e(nc.sbuf_tensor("irnd", [P, 2 * GS], mybir.dt.int32))
        ones_sb = e(nc.sbuf_tensor("ones", [P, 1], F32))
        S_sb = e(nc.sbuf_tensor("S_sb", [2 * T1G, CH_PER_CORE * BPAD], F32))
        svec = e(nc.sbuf_tensor("svec", [P, 2 * CH_PER_CORE], F32))
        hsum = e(nc.sbuf_tensor("hsum", [CH_PER_CORE, 1], F32))
        rinv = e(nc.sbuf_tensor("rinv", [CH_PER_CORE, 1], F32))
        packed = e(nc.sbuf_tensor("packed", [CH_PER_CORE, PACK], F32))
        gp = e(nc.sbuf_tensor("gp", [12, PACK], F32))
        gt = e(nc.sbuf_tensor("gt", [12, PACK], F32))
        t1s = e(nc.sbuf_tensor("t1s", [12, BINS], F32))
        t2s = e(nc.sbuf_tensor("t2s", [12, BINS], F32))
        ra = e(nc.sbuf_tensor("ra", [12, 1], F32))
        loss_sb = e(nc.sbuf_tensor("loss", [1, 1], F32))
        ps = e(nc.psum_tensor("ps", [2 * T1G, CH_PER_CORE * 2 * T2G], F32))
        ph2 = e(nc.psum_tensor("ph2", [CH_PER_CORE, BINS], F32))
        pl = e(nc.psum_tensor("pl", [1, 1], F32))
        dma_sem = e(nc.semaphore("dma_sem"))
        dve_pair = e(nc.semaphore("dve_pair"))
        act_pair = e(nc.semaphore("act_pair"))
        pe_strip = e(nc.semaphore("pe_strip"))
        pe_sem = e(nc.semaphore("pe_sem"))
        dve_sem = e(nc.semaphore("dve_sem"))
        act_sem = e(nc.semaphore("act_sem"))
        cc_sem = e(nc.semaphore("cc_sem"))
        block = e(nc.Block())

        def strip_x(s):
            c, si = divmod(s, NSTRIP_PER_CH)
            return xs[:, c * G + si * GS : c * G + (si + 1) * GS]

        @block.sync
        def _(sync: bass.BassEngine):
            sync.dma_start(out=consts[:, :], in_=consts_dram[:, :]).then_inc(
                dma_sem, 16
            )
            for c in range(CH_PER_CORE):
                sync.dma_start(
                    out=xs[:, c * G : (c + 1) * G], in_=x_ext[c, :, :]
                ).then_inc(dma_sem, 16)
            # S block -> DRAM -> svec (partition reshape)
            sync.wait_ge(dve_sem, 1)
            sync.dma_start(out=sdram[:, :, :], in_=S_sb[:, :]).then_inc(dma_sem, 16)
            for c in range(CH_PER_CORE):
                for h in range(2):
                    lo = h * 8
                    hi = min(2 * T1G, lo + 8)
                    sync.dma_start(
                        out=svec[0 : (hi - lo) * BPAD, 2 * c + h : 2 * c + h + 1],
                        in_=sdram[lo:hi, c, :],
                    ).then_inc(dma_sem, 16)
            # packed cdfs -> collective input
            sync.wait_ge(dve_sem, 2)
            sync.dma_start(out=cdf_in[:, :], in_=packed[:, :]).then_inc(dma_sem, 16)
            # gathered payload
            sync.wait_ge(cc_sem, 1)
            sync.dma_start(out=gp[:, :], in_=cdf_all[0:4, :]).then_inc(dma_sem, 16)
            sync.dma_start(out=gt[:, :], in_=cdf_all[4:8, :]).then_inc(dma_sem, 16)
            # final scalar
            sync.wait_ge(act_sem, 1)
            sync.dma_start(out=out_ext[:, :], in_=loss_sb[:, :]).then_inc(dma_sem, 16)

        @block.vector
        def _(vector: bass.BassVectorEngine):
            vector.memset(ones_sb[:, :], 1.0)
            vector.memset(svec[:, :], 0.0)
            vector.memset(S_sb[:, :], 0.0)
            # constant t=0 planes: cos=1, sin=0
            for b in range(2):
                vector.memset(plA[b][:, 0:GS], 1.0)
                vector.memset(plA[b][:, GS : 2 * GS], 0.0)
                vector.memset(plB[b][:, 0:GS], 1.0)
                vector.memset(plB[b][:, GS : 2 * GS], 0.0)
            vector.wait_ge(dma_sem, 64)
            for s in range(NSTRIPS):
                xin = strip_x(s)
                for k, (t, _, _) in enumerate(pairs):
                    kg = s * NPAIR + k
                    if kg >= 2:
                        vector.wait_ge(act_pair, kg - 1)
                    slot = (kg % 2) * 2
                    fr = args[:, slot * GS : (slot + 1) * GS]
                    frc = args[:, (slot + 1) * GS : (slot + 2) * GS]
                    ir = irnd[:, (kg % 2) * GS : (kg % 2 + 1) * GS]
                    # fr = t*0.8*x - round(t*0.8*x)  in [-0.5, 0.5]
                    vector.tensor_scalar(ir, xin, t * 0.8, None, OP.mult)
                    vector.scalar_tensor_tensor(
                        fr, xin, t * 0.8, ir, OP.mult, OP.subtract
                    )
                    # frc = (fr+0.25) - round(fr+0.25): phase shifted by +pi/2
                    vector.tensor_scalar(ir, fr, 0.25, None, OP.add)
                    vector.scalar_tensor_tensor(
                        frc, fr, 0.25, ir, OP.add, OP.subtract
                    ).then_inc(dve_pair, 1)
            # ---- after PE S-accumulation: normalize + cumsum + pack ----
            vector.wait_ge(pe_strip, NSTRIPS)
            for c in range(CH_PER_CORE):
                ins = vector.tensor_copy(
                    S_sb[:, c * BPAD : c * BPAD + 2 * T2G],
                    ps[:, c * 2 * T2G : (c + 1) * 2 * T2G],
                )
            ins.then_inc(dve_sem, 1)
            vector.wait_ge(pe_sem, 1)
            vector.tensor_reduce(hsum[:, :], ph2[:, :], AX.X, OP.add)
            vector.scalar_tensor_tensor(
                hsum[:, :], hsum[:, :], 1e-8, hsum[:, :], OP.add, OP.bypass
            )
            vector.reciprocal(rinv[:, :], hsum[:, :])
            vector.tensor_tensor_scan(
                packed[:, 0:BINS],
                ph2[:, :],
                t1s[0:CH_PER_CORE, :],
                0.0,
                OP.add,
                OP.bypass,
            )
            vector.tensor_copy(packed[:, BINS : BINS + 1], rinv[:, :]).then_inc(
                dve_sem, 1
            )
            # ---- loss stage ----
            vector.wait_ge(dma_sem, 224)
            vector.scalar_tensor_tensor(
                t1s[:, :],
                gp[:, 0:BINS],
                gp[:, BINS : BINS + 1],
                gp[:, 0:BINS],
                OP.mult,
                OP.bypass,
            )
            vector.scalar_tensor_tensor(
                t2s[:, :],
                gt[:, 0:BINS],
                gt[:, BINS : BINS + 1],
                t1s[:, :],
                OP.mult,
                OP.subtract,
            )
            vector.tensor_reduce(
                ra[:, :], t2s[:, :], AX.X, OP.add, apply_absolute_value=True
            ).then_inc(dve_sem, 1)

        @block.scalar
        def _(scalar: bass.BassScalarEngine):
            scalar.wait_ge(dma_sem, 16)
            TWO_PI = 2.0 * math.pi
            for s in range(NSTRIPS):
                b = s % 2
                if s >= 2:
                    scalar.wait_ge(pe_strip, s - 1)
                for k, (t, which, q) in enumerate(pairs):
                    kg = s * NPAIR + k
                    scalar.wait_ge(dve_pair, kg + 1)
                    slot = (kg % 2) * 2
                    fr = args[:, slot * GS : (slot + 1) * GS]
                    frc = args[:, (slot + 1) * GS : (slot + 2) * GS]
                    pl_ = plA[b] if which == "A" else plB[b]
                    # -cos into plane q, -sin into plane q+1
                    scalar.activation(
                        pl_[:, q * GS : (q + 1) * GS],
                        frc,
                        AF.Sin,
                        bias=consts[:, 0:1],
                        scale=TWO_PI,
                    )
                    scalar.activation(
                        pl_[:, (q + 1) * GS : (q + 2) * GS],
                        fr,
                        AF.Sin,
                        bias=consts[:, 0:1],
                        scale=TWO_PI,
                    ).then_inc(act_pair, 1)
            # final: loss = pl / 768
            scalar.wait_ge(pe_sem, 2)
            scalar.mul(loss_sb[:, :], pl[:, :], 1.0 / 768.0).then_inc(act_sem, 1)

        @block.tensor
        def _(tensor: bass.BassTensorEngine):
            for s in range(NSTRIPS):
                b = s % 2
                c, si = divmod(s, NSTRIP_PER_CH)
                tensor.wait_ge(act_pair, NPAIR * (s + 1))
                for g in range(GS):
                    ins = tensor.matmul(
                        ps[:, 2 * T2G * c : 2 * T2G * (c + 1)],
                        plB[b][:, g :: GS],
                        plA[b][:, g :: GS],
                        start=(si == 0 and g == 0),
                        stop=(si == NSTRIP_PER_CH - 1 and g == GS - 1),
                    )
                ins.then_inc(pe_strip, 1)
            # synthesis: ph2[c, j] = sum_h sum_p svec[p, 2c+h] * K[p, h*64+j]
            tensor.wait_ge(dma_sem, 176)
            for h in range(2):
                ins = tensor.matmul(
                    ph2[:, :],
                    svec[:, h :: 2],
                    consts[:, 1 + h * BINS : 1 + (h + 1) * BINS],
                    start=(h == 0),
                    stop=(h == 1),
                )
            ins.then_inc(pe_sem, 1)
            tensor.wait_ge(dve_sem, 3)
            tensor.matmul(
                pl[0:1, 0:1], ones_sb[0:12, 0:1], ra[0:12, 0:1], start=True, stop=True
            ).then_inc(pe_sem, 1)  # pe_sem reaches 2

        @block.gpsimd
        def _(gpsimd: bass.BassGpSimd):
            gpsimd.wait_ge(dma_sem, 192)
            gpsimd.collective_compute(
                "AllGather",
                OP.bypass,
                replica_groups=[list(range(N_CORES))],
                ins=[cdf_in.ap()],
                outs=[cdf_all.ap()],
            ).then_inc(cc_sem, 1)

    return nc


import os

_VERSION = int(os.environ.get("BASS_HIST_V", "2"))
_NC_CACHE = None


def _get_nc():
    global _NC_CACHE
    if _NC_CACHE is None:
        _NC_CACHE = build_nc_fourier() if _VERSION == 2 else build_nc()
    return _NC_CACHE


def kernel(pred: np.ndarray, target: np.ndarray) -> np.ndarray:
    assert pred.shape == (4, 3, 512, 512) and target.shape == (4, 3, 512, 512)
    chans = np.concatenate(
        [
            np.ascontiguousarray(pred, dtype=np.float32).reshape(12, P, G),
            np.ascontiguousarray(target, dtype=np.float32).reshape(12, P, G),
        ],
        axis=0,
    )  # [24, 128, 2048]

    in_maps = [
        {"x": np.ascontiguousarray(chans[3 * i : 3 * i + 3])} for i in range(N_CORES)
    ]

    nc = _get_nc()
    res = run_bass_kernel_spmd(nc, in_maps, core_ids=list(range(N_CORES)))
    out = res.results[0]["out"]
    return np.asarray(out, dtype=np.float32).reshape(())
